# revision 21
# baseline (speedup 1.0000x reference)
"""Trainium2 Bass kernel for the inverse-STFT decoder.

Computation (per batch element):
  frames = irfft(stft_real + i*stft_imag, n=512)        # [F, 512]
  frames *= inverse_stft_window(hann, frame=512, hop=128)
  sig = overlap_add(frames, hop=128)[: (F-1)*128 + 512 - 1]

Device strategy (pure data parallel, batch 16 -> 2 per core x 8 cores):
  - irfft+window is a matmul against a constant [512, 512] matrix W:
    row space = stacked (real bins 0..256, imag bins 1..255) -- irfft
    provably ignores Im(bin 0) and Im(bin 256) -- so the contraction is
    exactly 512 = 4 chunks of 128.
  - The host re-lays-out inputs to x[b, p, kc, f] = spectra^T so the
    contraction dim lands on SBUF partitions with no on-chip transpose.
  - Overlap-add folds into PSUM accumulation: output block tile
    o[n', blk] = sum_c sum_k W[k, 128c+n'] * xT[k, blk-c], i.e. 16
    matmuls (4 k-chunks x 4 shifts) per 512-block tile, fp32r at full
    PE rate (N=512 >= 256).
  - Device writes o[b, n', blk]; host transposes back to sample order.
"""

import numpy as np

import concourse.bacc as bacc
import concourse.mybir as mybir
import concourse.tile as tile
from concourse.bass_utils import run_bass_kernel_spmd

# Problem constants (hardcoded per harness contract)
B, FRAMES, BINS = 16, 4000, 257
FFT = 512
HOP = 128
N_CORES = 8
B_SH = B // N_CORES  # batch per core
NBLK = FRAMES - 1 + FFT // HOP  # 4003 output blocks of 128 samples
OUT_LEN = NBLK * HOP  # 512384; final output drops the last sample
BLK_TILE = 512  # output blocks per tile (one PSUM bank, max fp32 N)
# fp32r matmuls need even moving-N and even PSUM column offsets, so the
# frame axis is zero-padded (3 front for the c-shifts, 4 back to cover the
# padded block count) and the block axis padded to even 4004. Every matmul
# is then full-range at offset 0.
NBLK_PAD = 4004
F_PAD_LO = 3
FRAMES_PAD = F_PAD_LO + FRAMES + 4  # 4007

F32 = mybir.dt.float32
F32R = mybir.dt.float32r

# exec results of the last run (for test harness introspection)
LAST_RESULTS = None


def _inverse_window():
    n = np.arange(FFT, dtype=np.float64)
    win = 0.5 - 0.5 * np.cos(2.0 * np.pi * n / FFT)
    denom = win * win
    overlaps = -(-FFT // HOP)
    denom = np.pad(denom, (0, overlaps * HOP - FFT))
    denom = denom.reshape(overlaps, HOP).sum(axis=0)
    denom = np.tile(denom, overlaps)[:FFT]
    return win / denom


def _build_w_dev():
    """w_dev [128, 16, 128]: w_dev[p, kc*4+c, n'] = W[kc*128+p, 128c+n']."""
    n = np.arange(FFT)
    k = np.arange(BINS)
    C = np.cos(2 * np.pi * np.outer(k, n) / FFT) / FFT
    C[1:256] *= 2
    S = -np.sin(2 * np.pi * np.outer(k, n) / FFT) / FFT
    S[1:256] *= 2
    W = np.concatenate([C, S[1:256]], axis=0) * _inverse_window()[None, :]
    W = W.astype(np.float32)  # [512, 512]
    return np.ascontiguousarray(
        W.reshape(4, 128, 4, 128).transpose(1, 0, 2, 3).reshape(128, 16, 128)
    )


def build_nc():
    nc = bacc.Bacc(None, target_bir_lowering=False, debug=False)
    x = nc.dram_tensor(
        "x", [B_SH, 128, 4, FRAMES_PAD], F32R, kind="ExternalInput"
    ).ap()
    w = nc.dram_tensor("w", [128, 16, 128], F32R, kind="ExternalInput").ap()
    o = nc.dram_tensor("o", [B_SH, 128, NBLK_PAD], F32, kind="ExternalOutput").ap()

    n_tiles = -(-NBLK_PAD // BLK_TILE)

    with tile.TileContext(nc) as tc:
        with (
            tc.tile_pool(name="wpool", bufs=1) as wp,
            tc.tile_pool(name="xpool", bufs=3) as xp,
            tc.tile_pool(name="opool", bufs=3) as op,
            tc.tile_pool(name="psum", bufs=2, space="PSUM") as pp,
        ):
            wt = wp.tile([128, 16, 128], F32R)
            nc.sync.dma_start(wt[:], w[:])

            for b in range(B_SH):
                for t in range(n_tiles):
                    B0 = BLK_TILE * t
                    NB = min(BLK_TILE, NBLK_PAD - B0)
                    NF = NB + 3  # 3-frame halo for the c-shifts

                    xt = xp.tile([128, 4, NF], F32R)
                    nc.sync.dma_start(xt[:], x[b, :, :, B0 : B0 + NF])

                    pt = pp.tile([128, NB], F32)
                    for mm in range(16):
                        c, kc = divmod(mm, 4)
                        # out block B0+i sums W[:,128c:128c+128]^T @
                        # xT[:, B0+i-c]; padded frame col j = i + 3 - c
                        nc.tensor.matmul(
                            pt[:],
                            lhsT=wt[:, kc * 4 + c, :],
                            rhs=xt[:, kc, 3 - c : 3 - c + NB],
                            start=(mm == 0),
                            stop=(mm == 15),
                        )

                    ot = op.tile([128, NB], F32)
                    nc.scalar.copy(ot[:], pt[:])
                    nc.sync.dma_start(o[b, :, B0 : B0 + NB], ot[:])

    nc.compile()
    return nc


def _pack_inputs(stft_real, stft_imag):
    """-> x_dev [B, 128, 4, FRAMES_PAD] f32 (transposed, stacked, padded)."""
    xt = np.concatenate(
        [stft_real.transpose(0, 2, 1), stft_imag[:, :, 1:256].transpose(0, 2, 1)],
        axis=1,
    )  # [B, 512, F]
    xt = xt.reshape(B, 4, 128, FRAMES).transpose(0, 2, 1, 3)  # [B, 128, 4, F]
    out = np.zeros((B, 128, 4, FRAMES_PAD), np.float32)
    out[:, :, :, F_PAD_LO : F_PAD_LO + FRAMES] = xt
    return out


def kernel(stft_real: np.ndarray, stft_imag: np.ndarray) -> np.ndarray:
    global LAST_RESULTS
    stft_real = np.ascontiguousarray(stft_real, dtype=np.float32)
    stft_imag = np.ascontiguousarray(stft_imag, dtype=np.float32)

    x_dev = _pack_inputs(stft_real, stft_imag)
    w_dev = _build_w_dev()

    nc = build_nc()
    core_ids = list(range(N_CORES))
    in_maps = [
        {"x": x_dev[B_SH * i : B_SH * (i + 1)], "w": w_dev} for i in core_ids
    ]
    res = run_bass_kernel_spmd(nc, in_maps, core_ids)
    LAST_RESULTS = res

    out = np.empty((B, OUT_LEN - 1), np.float32)
    for i in core_ids:
        o = res.results[i]["o"]  # [B_SH, 128, NBLK_PAD]
        sig = o.transpose(0, 2, 1).reshape(B_SH, NBLK_PAD * HOP)[:, : OUT_LEN - 1]
        out[B_SH * i : B_SH * (i + 1)] = sig
    return out


# revision 34
# speedup vs baseline: 237.1074x; 237.1074x over previous
"""Trainium2 Bass kernel for the inverse-STFT decoder.

Computation (per batch element):
  frames = irfft(stft_real + i*stft_imag, n=512)        # [F, 512]
  frames *= inverse_stft_window(hann, frame=512, hop=128)
  sig = overlap_add(frames, hop=128)[: (F-1)*128 + 512 - 1]

Device strategy (pure data parallel, batch 16 -> 2 per core x 8 cores):
  - irfft+window is a matmul against a constant [512, 512] matrix W:
    row space = stacked (real bins 0..256, imag bins 1..255) -- irfft
    provably ignores Im(bin 0) and Im(bin 256) -- so the contraction is
    exactly 512 = 4 chunks of 128.
  - The host re-lays-out inputs to x[b, p, kc, f] = spectra^T so the
    contraction dim lands on SBUF partitions with no on-chip transpose.
  - Overlap-add folds into PSUM accumulation: output block tile
    o[n', blk] = sum_c sum_k W[k, 128c+n'] * xT[k, blk-c], i.e. 16
    matmuls (4 k-chunks x 4 shifts) per 512-block tile, fp32r at full
    PE rate (N=512 >= 256).
  - Device writes o[b, n', blk]; host transposes back to sample order.
"""

import contextlib
import os

import numpy as np

import concourse.bacc as bacc
import concourse.mybir as mybir
import concourse.tile as tile
from concourse.bass_utils import run_bass_kernel_spmd

# Problem constants (hardcoded per harness contract)
B, FRAMES, BINS = 16, 4000, 257
FFT = 512
HOP = 128
N_CORES = 8
B_SH = B // N_CORES  # batch per core
NBLK = FRAMES - 1 + FFT // HOP  # 4003 output blocks of 128 samples
OUT_LEN = NBLK * HOP  # 512384; final output drops the last sample
BLK_TILE = 512  # output blocks per tile (one PSUM bank, max fp32 N)
# fp32r matmuls need even moving-N and even PSUM column offsets, so the
# frame axis is zero-padded (3 front for the c-shifts, 4 back to cover the
# padded block count) and the block axis padded to even 4004. Every matmul
# is then full-range at offset 0.
NBLK_PAD = 4004
F_PAD_LO = 3
FRAMES_PAD = F_PAD_LO + FRAMES + 4  # 4007

F32 = mybir.dt.float32
F32R = mybir.dt.float32r

# exec results of the last run (for test harness introspection)
LAST_RESULTS = None


def _inverse_window():
    n = np.arange(FFT, dtype=np.float64)
    win = 0.5 - 0.5 * np.cos(2.0 * np.pi * n / FFT)
    denom = win * win
    overlaps = -(-FFT // HOP)
    denom = np.pad(denom, (0, overlaps * HOP - FFT))
    denom = denom.reshape(overlaps, HOP).sum(axis=0)
    denom = np.tile(denom, overlaps)[:FFT]
    return win / denom


def _build_w_dev():
    """w_dev [128, 16, 128]: w_dev[p, kc*4+c, n'] = W[kc*128+p, 128c+n']."""
    n = np.arange(FFT)
    k = np.arange(BINS)
    C = np.cos(2 * np.pi * np.outer(k, n) / FFT) / FFT
    C[1:256] *= 2
    S = -np.sin(2 * np.pi * np.outer(k, n) / FFT) / FFT
    S[1:256] *= 2
    W = np.concatenate([C, S[1:256]], axis=0) * _inverse_window()[None, :]
    W = W.astype(np.float32)  # [512, 512]
    return np.ascontiguousarray(
        W.reshape(4, 128, 4, 128).transpose(1, 0, 2, 3).reshape(128, 16, 128)
    )


def build_nc(
    reps: int = 1,
    xbufs: int = 3,
    obufs: int = 3,
    pbufs: int = 2,
    loop_reps: int = 0,
    nsplit_x: int = 1,
    evict: str = "act",
):
    """loop_reps>0 wraps the whole computation in a hardware For_i loop that
    repeats it that many times -- used only for timing amplification."""
    nc = bacc.Bacc(None, target_bir_lowering=False, debug=False)
    x = nc.dram_tensor(
        "x", [B_SH, 128, 4, FRAMES_PAD], F32R, kind="ExternalInput"
    ).ap()
    w = nc.dram_tensor("w", [128, 16, 128], F32R, kind="ExternalInput").ap()
    o = nc.dram_tensor("o", [B_SH, 128, NBLK_PAD], F32, kind="ExternalOutput").ap()

    n_tiles = -(-NBLK_PAD // BLK_TILE)

    with tile.TileContext(nc) as tc:
        with (
            tc.tile_pool(name="wpool", bufs=1) as wp,
            tc.tile_pool(name="xpool", bufs=xbufs) as xp,
            tc.tile_pool(name="opool", bufs=obufs) as op,
            tc.tile_pool(name="psum", bufs=pbufs, space="PSUM") as pp,
        ):
            wt = wp.tile([128, 16, 128], F32R)
            nc.sync.dma_start(wt[:], w[:])

            loop_cm = (
                tc.For_i(0, loop_reps, 1, hint_engines=(mybir.EngineType.PE,))
                if loop_reps > 0
                else contextlib.nullcontext()
            )
            with loop_cm:
              for _rep in range(reps):
                for b in range(B_SH):
                  for t in range(n_tiles):
                    B0 = BLK_TILE * t
                    NB = min(BLK_TILE, NBLK_PAD - B0)
                    NF = NB + 3  # 3-frame halo for the c-shifts

                    xt = xp.tile([128, 4, NF], F32R)
                    kstep = 4 // nsplit_x
                    for s in range(nsplit_x):
                        nc.sync.dma_start(
                            xt[:, s * kstep : (s + 1) * kstep],
                            x[b, :, s * kstep : (s + 1) * kstep, B0 : B0 + NF],
                        )

                    pt = pp.tile([128, NB], F32)
                    for mm in range(16):
                        c, kc = divmod(mm, 4)
                        # out block B0+i sums W[:,128c:128c+128]^T @
                        # xT[:, B0+i-c]; padded frame col j = i + 3 - c
                        nc.tensor.matmul(
                            pt[:],
                            lhsT=wt[:, kc * 4 + c, :],
                            rhs=xt[:, kc, 3 - c : 3 - c + NB],
                            start=(mm == 0),
                            stop=(mm == 15),
                        )

                    ot = op.tile([128, NB], F32)
                    if evict == "act":
                        nc.scalar.copy(ot[:], pt[:])
                    else:
                        nc.vector.tensor_copy(ot[:], pt[:])
                    nc.sync.dma_start(o[b, :, B0 : B0 + NB], ot[:])

    nc.compile()
    return nc


def _pack_inputs(stft_real, stft_imag):
    """-> x_dev [B, 128, 4, FRAMES_PAD] f32 (transposed, stacked, padded)."""
    xt = np.concatenate(
        [stft_real.transpose(0, 2, 1), stft_imag[:, :, 1:256].transpose(0, 2, 1)],
        axis=1,
    )  # [B, 512, F]
    xt = xt.reshape(B, 4, 128, FRAMES).transpose(0, 2, 1, 3)  # [B, 128, 4, F]
    out = np.zeros((B, 128, 4, FRAMES_PAD), np.float32)
    out[:, :, :, F_PAD_LO : F_PAD_LO + FRAMES] = xt
    return out


def kernel(stft_real: np.ndarray, stft_imag: np.ndarray) -> np.ndarray:
    global LAST_RESULTS
    stft_real = np.ascontiguousarray(stft_real, dtype=np.float32)
    stft_imag = np.ascontiguousarray(stft_imag, dtype=np.float32)

    x_dev = _pack_inputs(stft_real, stft_imag)
    w_dev = _build_w_dev()

    nc = build_nc()
    core_ids = list(range(N_CORES))
    in_maps = [
        {"x": x_dev[B_SH * i : B_SH * (i + 1)], "w": w_dev} for i in core_ids
    ]
    try:
        res = run_bass_kernel_spmd(nc, in_maps, core_ids)
    except ModuleNotFoundError:
        # BASS_TRACE=1 on a bare axon client lacks antenv.axon_hooks;
        # retry with tracing off rather than failing the run.
        os.environ["BASS_NEVER_TRACE"] = "1"
        res = run_bass_kernel_spmd(nc, in_maps, core_ids)
    LAST_RESULTS = res

    out = np.empty((B, OUT_LEN - 1), np.float32)
    for i in core_ids:
        o = res.results[i]["o"]  # [B_SH, 128, NBLK_PAD]
        sig = o.transpose(0, 2, 1).reshape(B_SH, NBLK_PAD * HOP)[:, : OUT_LEN - 1]
        out[B_SH * i : B_SH * (i + 1)] = sig
    return out


# revision 57
# speedup vs baseline: 285.8150x; 1.2054x over previous
"""Trainium2 Bass kernel for the inverse-STFT decoder.

Computation (per batch element):
  frames = irfft(stft_real + i*stft_imag, n=512)        # [F, 512]
  frames *= inverse_stft_window(hann, frame=512, hop=128)
  sig = overlap_add(frames, hop=128)[: (F-1)*128 + 512 - 1]

Device strategy (pure data parallel, batch 16 -> 2 per core x 8 cores):
  - irfft+window is a matmul against a constant [512, 512] matrix W:
    row space = stacked (real bins 0..256, imag bins 1..255) -- irfft
    provably ignores Im(bin 0) and Im(bin 256) -- so the contraction is
    exactly 512 = 4 chunks of 128.
  - The host re-lays-out inputs to x[b, p, kc, f] = spectra^T so the
    contraction dim lands on SBUF partitions with no on-chip transpose.
  - Overlap-add folds into PSUM accumulation: output block tile
    o[n', blk] = sum_c sum_k W[k, 128c+n'] * xT[k, blk-c], i.e. 16
    matmuls (4 k-chunks x 4 shifts) per 512-block tile, fp32r at full
    PE rate (N=512 >= 256).
  - Device writes o[b, n', blk]; host transposes back to sample order.
"""

import contextlib
import os

import numpy as np

import concourse.bacc as bacc
import concourse.mybir as mybir
import concourse.tile as tile
from concourse.bass_utils import run_bass_kernel_spmd

# Problem constants (hardcoded per harness contract)
B, FRAMES, BINS = 16, 4000, 257
FFT = 512
HOP = 128
N_CORES = 8
B_SH = B // N_CORES  # batch per core
NBLK = FRAMES - 1 + FFT // HOP  # 4003 output blocks of 128 samples
OUT_LEN = NBLK * HOP  # 512384; final output drops the last sample
BLK_TILE = 512  # output blocks per tile (one PSUM bank, max fp32 N)
# fp32r matmuls need even moving-N and even PSUM column offsets, so the
# frame axis is zero-padded (3 front for the c-shifts, 4 back to cover the
# padded block count) and the block axis padded to even 4004. Every matmul
# is then full-range at offset 0.
NBLK_PAD = 4004
F_PAD_LO = 3
FRAMES_PAD = F_PAD_LO + FRAMES + 4  # 4007

F32 = mybir.dt.float32
F32R = mybir.dt.float32r
BF16 = mybir.dt.bfloat16

# KBF16=1 switches the datapath (x, w, o) to bf16: ~2x less DMA traffic at
# ~2.5e-3 rel err instead of 1.5e-4. Default is the precision-safe fp32r.
USE_BF16 = os.environ.get("KBF16") == "1"
DT_X = BF16 if USE_BF16 else F32R
DT_O = BF16 if USE_BF16 else F32

# exec results of the last run (for test harness introspection)
LAST_RESULTS = None


def _inverse_window():
    n = np.arange(FFT, dtype=np.float64)
    win = 0.5 - 0.5 * np.cos(2.0 * np.pi * n / FFT)
    denom = win * win
    overlaps = -(-FFT // HOP)
    denom = np.pad(denom, (0, overlaps * HOP - FFT))
    denom = denom.reshape(overlaps, HOP).sum(axis=0)
    denom = np.tile(denom, overlaps)[:FFT]
    return win / denom


def _build_w_dev():
    """w_dev [128, 16, 128]: w_dev[p, kc*4+c, n'] = W[kc*128+p, 128c+n']."""
    n = np.arange(FFT)
    k = np.arange(BINS)
    C = np.cos(2 * np.pi * np.outer(k, n) / FFT) / FFT
    C[1:256] *= 2
    S = -np.sin(2 * np.pi * np.outer(k, n) / FFT) / FFT
    S[1:256] *= 2
    W = np.concatenate([C, S[1:256]], axis=0) * _inverse_window()[None, :]
    W = W.astype(mybir.dt.np(DT_X))  # [512, 512]
    return np.ascontiguousarray(
        W.reshape(4, 128, 4, 128).transpose(1, 0, 2, 3).reshape(128, 16, 128)
    )


def build_nc(
    reps: int = 1,
    xbufs: int = 3,
    obufs: int = 3,
    pbufs: int = 2,
    loop_reps: int = 0,
    nsplit_x: int = 1,
    evict: str = "act",
):
    """loop_reps>0 wraps the whole computation in a hardware For_i loop that
    repeats it that many times -- used only for timing amplification."""
    nc = bacc.Bacc(None, target_bir_lowering=False, debug=False)
    x = nc.dram_tensor(
        "x", [B_SH, 128, 4, FRAMES_PAD], DT_X, kind="ExternalInput"
    ).ap()
    w = nc.dram_tensor("w", [128, 16, 128], DT_X, kind="ExternalInput").ap()
    o = nc.dram_tensor("o", [B_SH, 128, NBLK_PAD], DT_O, kind="ExternalOutput").ap()

    n_tiles = -(-NBLK_PAD // BLK_TILE)

    with tile.TileContext(nc) as tc:
        with (
            tc.tile_pool(name="wpool", bufs=1) as wp,
            tc.tile_pool(name="xpool", bufs=xbufs) as xp,
            tc.tile_pool(name="opool", bufs=obufs) as op,
            tc.tile_pool(name="psum", bufs=pbufs, space="PSUM") as pp,
        ):
            wt = wp.tile([128, 16, 128], DT_X)
            nc.sync.dma_start(wt[:], w[:])

            loop_cm = (
                tc.For_i(0, loop_reps, 1, hint_engines=(mybir.EngineType.PE,))
                if loop_reps > 0
                else contextlib.nullcontext()
            )
            with loop_cm:
              for _rep in range(reps):
                for b in range(B_SH):
                  for t in range(n_tiles):
                    B0 = BLK_TILE * t
                    NB = min(BLK_TILE, NBLK_PAD - B0)
                    NF = NB + 3  # 3-frame halo for the c-shifts

                    xt = xp.tile([128, 4, NF], DT_X)
                    kstep = 4 // nsplit_x
                    for s in range(nsplit_x):
                        nc.sync.dma_start(
                            xt[:, s * kstep : (s + 1) * kstep],
                            x[b, :, s * kstep : (s + 1) * kstep, B0 : B0 + NF],
                        )

                    pt = pp.tile([128, NB], F32)
                    for mm in range(16):
                        c, kc = divmod(mm, 4)
                        # out block B0+i sums W[:,128c:128c+128]^T @
                        # xT[:, B0+i-c]; padded frame col j = i + 3 - c
                        nc.tensor.matmul(
                            pt[:],
                            lhsT=wt[:, kc * 4 + c, :],
                            rhs=xt[:, kc, 3 - c : 3 - c + NB],
                            start=(mm == 0),
                            stop=(mm == 15),
                        )

                    ot = op.tile([128, NB], DT_O)
                    if evict == "act":
                        nc.scalar.copy(ot[:], pt[:])
                    else:
                        nc.vector.tensor_copy(ot[:], pt[:])
                    nc.sync.dma_start(o[b, :, B0 : B0 + NB], ot[:])

    nc.compile()
    return nc


def _pack_inputs(stft_real, stft_imag):
    """-> x_dev [B, 128, 4, FRAMES_PAD] f32 (transposed, stacked, padded)."""
    xt = np.concatenate(
        [stft_real.transpose(0, 2, 1), stft_imag[:, :, 1:256].transpose(0, 2, 1)],
        axis=1,
    )  # [B, 512, F]
    xt = xt.reshape(B, 4, 128, FRAMES).transpose(0, 2, 1, 3)  # [B, 128, 4, F]
    np_dt = mybir.dt.np(DT_X)
    out = np.zeros((B, 128, 4, FRAMES_PAD), np_dt)
    out[:, :, :, F_PAD_LO : F_PAD_LO + FRAMES] = xt.astype(np_dt)
    return out


def kernel(stft_real: np.ndarray, stft_imag: np.ndarray) -> np.ndarray:
    global LAST_RESULTS
    stft_real = np.ascontiguousarray(stft_real, dtype=np.float32)
    stft_imag = np.ascontiguousarray(stft_imag, dtype=np.float32)

    x_dev = _pack_inputs(stft_real, stft_imag)
    w_dev = _build_w_dev()

    nc = build_nc()
    core_ids = list(range(N_CORES))
    in_maps = [
        {"x": x_dev[B_SH * i : B_SH * (i + 1)], "w": w_dev} for i in core_ids
    ]
    try:
        res = run_bass_kernel_spmd(nc, in_maps, core_ids)
    except ModuleNotFoundError:
        # BASS_TRACE=1 on a bare axon client lacks antenv.axon_hooks;
        # retry with tracing off rather than failing the run.
        os.environ["BASS_NEVER_TRACE"] = "1"
        res = run_bass_kernel_spmd(nc, in_maps, core_ids)
    LAST_RESULTS = res

    out = np.empty((B, OUT_LEN - 1), np.float32)
    for i in core_ids:
        o = res.results[i]["o"].astype(np.float32)  # [B_SH, 128, NBLK_PAD]
        sig = o.transpose(0, 2, 1).reshape(B_SH, NBLK_PAD * HOP)[:, : OUT_LEN - 1]
        out[B_SH * i : B_SH * (i + 1)] = sig
    return out
